# revision 13
# baseline (speedup 1.0000x reference)
"""Ragged-sequence attention kernel for 8 Trainium2 NeuronCores.

Problem: N=64 batches, T=2048, D=256.
  energy[n,t] = <key[n,t,:], query[n,:]>
  att = softmax(energy) masked to t < lens[n], renormalized
  context[n,:] = sum_t att[n,t] * value[n,t,:]

Math identity used: global softmax + mask + renormalize == masked softmax
(the global denominator cancels), so we compute exp(e - 20) with an additive
-1e9 mask folded into the energy, then normalize by the masked sum.

Sharding: pure data parallel, batch dim split 8 ways (8 batches/core).
"""

import numpy as np

N, T, D = 64, 2048, 256
NCORES = 8
NB = N // NCORES          # 8 batches per core
P = 128                   # SBUF partitions
CHUNKS = T // P           # 16 chunks of 128 timesteps
HALF = CHUNKS // 2        # key/value loaded in two half-DMAs

_CACHE = {}


def _build():
    import concourse.bass as bass
    import concourse.tile as tile
    from concourse import mybir

    f32 = mybir.dt.float32
    Alu = mybir.AluOpType

    nc = bass.Bass()

    q_d = nc.declare_dram_parameter("query", [NB, D], f32, isOutput=False)
    k_d = nc.declare_dram_parameter("key", [NB, T, D], f32, isOutput=False)
    v_d = nc.declare_dram_parameter("value", [NB, T, D], f32, isOutput=False)
    m_d = nc.declare_dram_parameter("addmask", [NB, P, CHUNKS], f32, isOutput=False)
    id_d = nc.declare_dram_parameter("ident", [P, P], f32, isOutput=False)
    ctx_d = nc.declare_dram_parameter("ctx_out", [NB, D], f32, isOutput=True)
    att_d = nc.declare_dram_parameter("att_out", [NB, T], f32, isOutput=True)

    # t = c*128 + p  ->  partition p, chunk c
    key4 = k_d[:].rearrange("n (c p) d -> n p c d", p=P)
    val4 = v_d[:].rearrange("n (c p) d -> n p c d", p=P)
    att3 = att_d[:].rearrange("n (c p) -> n c p", p=P)

    with tile.TileContext(nc) as tc:
        with (
            tc.tile_pool(name="const", bufs=1) as constp,
            tc.tile_pool(name="kv", bufs=4) as kvp,
            tc.tile_pool(name="small", bufs=3) as smp,
            tc.tile_pool(name="scratch", bufs=2) as scp,
            tc.tile_pool(name="ps", bufs=8, space="PSUM") as psp,
        ):
            ones_col = constp.tile([P, 1], f32)
            nc.vector.memset(ones_col[:], 1.0)
            ones_row = constp.tile([1, P], f32)
            nc.vector.memset(ones_row[:], 1.0)
            ident = constp.tile([P, P], f32)
            nc.sync.dma_start(ident[:], id_d[:])
            bias_sb = constp.tile([P, 1], f32)
            nc.vector.memset(bias_sb[:], -20.0)

            for n in range(NB):
                # q broadcast across partitions via stride-0 DMA
                q_rep = smp.tile([P, D], f32, tag="q_rep")
                nc.sync.dma_start(q_rep[:], q_d[n : n + 1, :].to_broadcast((P, D)))
                mask_sb = smp.tile([P, CHUNKS], f32, tag="mask")
                nc.sync.dma_start(mask_sb[:], m_d[n])

                QTR = HALF // 2
                kts, vts = [], []
                for h in range(2):
                    kt = kvp.tile([P, HALF, D], f32, tag=f"kt{h}")
                    for q4 in range(2):
                        lo = h * HALF + q4 * QTR
                        nc.sync.dma_start(
                            kt[:, q4 * QTR : (q4 + 1) * QTR, :],
                            key4[n, :, lo : lo + QTR, :],
                        )
                    kts.append(kt)
                for h in range(2):
                    vt = kvp.tile([P, HALF, D], f32, tag=f"vt{h}")
                    for q4 in range(2):
                        lo = h * HALF + q4 * QTR
                        nc.sync.dma_start(
                            vt[:, q4 * QTR : (q4 + 1) * QTR, :],
                            val4[n, :, lo : lo + QTR, :],
                        )
                    vts.append(vt)

                # energy e[p, c] = sum_d key[p, c, d] * q[d]
                # DVE: broadcast multiply per half; reduction split between
                # DVE (one 3D reduce for half 0) and ACT (copy+accum, half 1)
                e_sb = smp.tile([P, CHUNKS], f32, tag="e_sb")
                q_bcast = q_rep[:][:, None, :].to_broadcast((P, HALF, D))
                prods = []
                for h in range(2):
                    prod = scp.tile([P, HALF, D], f32, tag=f"scr{h}")
                    nc.vector.tensor_tensor(prod[:], kts[h][:], q_bcast, Alu.mult)
                    prods.append(prod)
                nc.vector.tensor_reduce(
                    e_sb[:, 0:HALF], prods[0][:], mybir.AxisListType.X, Alu.add
                )
                for c in range(HALF):
                    trash = scp.tile([P, D], f32, tag="trash", bufs=1)
                    nc.scalar.activation(
                        trash[:],
                        prods[1][:, c, :],
                        mybir.ActivationFunctionType.Copy,
                        accum_out=e_sb[:, HALF + c : HALF + c + 1],
                    )

                # additive mask (-1e9 at t >= lens), then exp with fused row-sum
                em_sb = smp.tile([P, CHUNKS], f32, tag="em_sb")
                nc.vector.tensor_tensor(em_sb[:], e_sb[:], mask_sb[:], Alu.add)
                p_sb = smp.tile([P, CHUNKS], f32, tag="p_sb")
                part_sb = smp.tile([P, 1], f32, tag="part")
                nc.scalar.activation(
                    p_sb[:],
                    em_sb[:],
                    mybir.ActivationFunctionType.Exp,
                    bias=bias_sb[:],
                    scale=1.0,
                    accum_out=part_sb[:],
                )

                # total s = sum over partitions of part, r = 1/s
                s_ps = psp.tile([1, 1], f32, tag="ps")
                nc.tensor.matmul(s_ps[:], ones_col[:], part_sb[:])
                r_sb = smp.tile([1, 1], f32, tag="r_sb")
                nc.vector.reciprocal(r_sb[:], s_ps[:])

                # context[d] = r * sum_t p[t] * value[t, d] (accumulate over chunks)
                ctx_ps = psp.tile([1, D], f32, tag="ps")
                for c in range(CHUNKS):
                    nc.tensor.matmul(
                        ctx_ps[:],
                        p_sb[:, c : c + 1],
                        vts[c // HALF][:, c % HALF, :],
                        start=(c == 0),
                        stop=(c == CHUNKS - 1),
                    )
                ctx_sb = smp.tile([1, D], f32, tag="ctx_sb")
                nc.scalar.mul(ctx_sb[:], ctx_ps[:], mul=r_sb[:])
                nc.sync.dma_start(ctx_d[n : n + 1, :], ctx_sb[:])

                # attention out: transpose p to [c, p] layout, scale by r, store
                pT_ps = psp.tile([CHUNKS, P], f32, tag="ps")
                nc.tensor.transpose(pT_ps[:], p_sb[:], ident[:])
                rb_ps = psp.tile([CHUNKS, 1], f32, tag="ps")
                nc.tensor.matmul(rb_ps[:], ones_row[0:1, 0:CHUNKS], r_sb[:])
                rb_sb = smp.tile([CHUNKS, 1], f32, tag="rb_sb")
                nc.vector.tensor_copy(rb_sb[:], rb_ps[:])
                att_sb = smp.tile([CHUNKS, P], f32, tag="att_sb")
                nc.scalar.mul(att_sb[:], pT_ps[:], mul=rb_sb[:])
                nc.sync.dma_start(att3[n], att_sb[:])

    return nc


def _split_multiwaits(bir):
    """The walrus build in this container allows only ONE sync wait per
    instruction (setupSyncWait: 'Too many sync wait commands'). Tile attaches
    multiple waits to single instructions. Split the extras into standalone
    EventSemaphore wait instructions (same engine, placed immediately before)
    — semantically identical, just sequential waits."""
    ctr = 0
    for fn in bir["functions"]:
        for blk in fn["blocks"]:
            out = []
            for inst in blk["instructions"]:
                si = inst.get("sync_info")
                waits = (si or {}).get("on_wait") or []
                # raw-bytes ISA instructions encode waits in their payload;
                # rewriting sync_info desyncs the encoded length
                if len(waits) > 1 and not inst.get("instr"):
                    for w in waits[:-1]:
                        ctr += 1
                        pre = {
                            "name": f"I-mw{ctr}",
                            "opcode": "EventSemaphore",
                            "engine": inst["engine"],
                            "ins": [],
                            "outs": [],
                            "sync_info": {"on_update": [], "on_wait": [w]},
                        }
                        if "debug" in inst:
                            pre["debug"] = inst["debug"]
                        out.append(pre)
                    si["on_wait"] = [waits[-1]]
                out.append(inst)
            blk["instructions"] = out
    return bir


def _patch_json(nc):
    import json as _json

    orig = nc.to_json_bytes

    def patched():
        bir = _json.loads(orig())
        _split_multiwaits(bir)
        return _json.dumps(bir).encode()

    nc.to_json_bytes = patched
    return nc


def _get_nc():
    if "nc" not in _CACHE:
        _CACHE["nc"] = _patch_json(_build())
    return _CACHE["nc"]


def _make_in_maps(query, key, value, lens):
    query = np.ascontiguousarray(np.asarray(query, dtype=np.float32))
    key = np.ascontiguousarray(np.asarray(key, dtype=np.float32))
    value = np.ascontiguousarray(np.asarray(value, dtype=np.float32))
    lens = np.asarray(lens).astype(np.int64)

    # addmask[n, p, c] = 0 if c*128+p < lens[n] else -1e9
    t_idx = (np.arange(CHUNKS)[None, :] * P + np.arange(P)[:, None]).astype(np.int64)
    addmask = np.where(t_idx[None, :, :] < lens[:, None, None], 0.0, -1e9).astype(
        np.float32
    )
    ident = np.eye(P, dtype=np.float32)

    in_maps = []
    for i in range(NCORES):
        sl = slice(i * NB, (i + 1) * NB)
        in_maps.append(
            {
                "query": query[sl],
                "key": key[sl],
                "value": value[sl],
                "addmask": np.ascontiguousarray(addmask[sl]),
                "ident": ident,
            }
        )
    return in_maps


def _run(in_maps, trace=False, **kwargs):
    from concourse.bass_utils import run_bass_kernel_spmd

    nc = _get_nc()
    return run_bass_kernel_spmd(
        nc, in_maps, core_ids=list(range(NCORES)), trace=trace, **kwargs
    )


def kernel(query, key, value, lens):
    in_maps = _make_in_maps(query, key, value, lens)
    res = _run(in_maps, trace=False)
    ctx = np.concatenate([r["ctx_out"] for r in res.results], axis=0)
    att = np.concatenate([r["att_out"] for r in res.results], axis=0)
    return ctx, att


# revision 14
# speedup vs baseline: 1.1647x; 1.1647x over previous
"""Ragged-sequence attention kernel for 8 Trainium2 NeuronCores.

Problem: N=64 batches, T=2048, D=256.
  energy[n,t] = <key[n,t,:], query[n,:]>
  att = softmax(energy) masked to t < lens[n], renormalized
  context[n,:] = sum_t att[n,t] * value[n,t,:]

Math identity used: global softmax + mask + renormalize == masked softmax
(the global denominator cancels), so we compute exp(e - 20) with an additive
-1e9 mask folded into the energy, then normalize by the masked sum.

Sharding: pure data parallel, batch dim split 8 ways (8 batches/core).
"""

import numpy as np

N, T, D = 64, 2048, 256
NCORES = 8
NB = N // NCORES          # 8 batches per core
P = 128                   # SBUF partitions
CHUNKS = T // P           # 16 chunks of 128 timesteps
HALF = CHUNKS // 2        # key/value loaded in two half-DMAs

_CACHE = {}


def _build():
    import concourse.bass as bass
    import concourse.tile as tile
    from concourse import mybir

    f32 = mybir.dt.float32
    Alu = mybir.AluOpType

    nc = bass.Bass()

    q_d = nc.declare_dram_parameter("query", [NB, D], f32, isOutput=False)
    k_d = nc.declare_dram_parameter("key", [NB, T, D], f32, isOutput=False)
    v_d = nc.declare_dram_parameter("value", [NB, T, D], f32, isOutput=False)
    m_d = nc.declare_dram_parameter("addmask", [NB, P, CHUNKS], f32, isOutput=False)
    id_d = nc.declare_dram_parameter("ident", [P, P], f32, isOutput=False)
    ctx_d = nc.declare_dram_parameter("ctx_out", [NB, D], f32, isOutput=True)
    att_d = nc.declare_dram_parameter("att_out", [NB, T], f32, isOutput=True)

    # t = c*128 + p  ->  partition p, chunk c
    key4 = k_d[:].rearrange("n (c p) d -> n p c d", p=P)
    val4 = v_d[:].rearrange("n (c p) d -> n p c d", p=P)
    att3 = att_d[:].rearrange("n (c p) -> n c p", p=P)

    with tile.TileContext(nc) as tc:
        with (
            tc.tile_pool(name="const", bufs=1) as constp,
            tc.tile_pool(name="kv", bufs=4) as kvp,
            tc.tile_pool(name="small", bufs=3) as smp,
            tc.tile_pool(name="scratch", bufs=2) as scp,
            tc.tile_pool(name="ps", bufs=8, space="PSUM") as psp,
        ):
            ones_col = constp.tile([P, 1], f32)
            nc.vector.memset(ones_col[:], 1.0)
            ones_row = constp.tile([1, P], f32)
            nc.vector.memset(ones_row[:], 1.0)
            ident = constp.tile([P, P], f32)
            nc.sync.dma_start(ident[:], id_d[:])
            bias_sb = constp.tile([P, 1], f32)
            nc.vector.memset(bias_sb[:], -20.0)

            for n in range(NB):
                # q broadcast across partitions via stride-0 DMA
                q_rep = smp.tile([P, D], f32, tag="q_rep")
                nc.sync.dma_start(q_rep[:], q_d[n : n + 1, :].to_broadcast((P, D)))
                mask_sb = smp.tile([P, CHUNKS], f32, tag="mask")
                nc.sync.dma_start(mask_sb[:], m_d[n])

                kts, vts = [], []
                for h in range(2):
                    kt = kvp.tile([P, HALF, D], f32, tag=f"kt{h}")
                    nc.sync.dma_start(kt[:], key4[n, :, h * HALF : (h + 1) * HALF, :])
                    kts.append(kt)
                for h in range(2):
                    vt = kvp.tile([P, HALF, D], f32, tag=f"vt{h}")
                    nc.sync.dma_start(vt[:], val4[n, :, h * HALF : (h + 1) * HALF, :])
                    vts.append(vt)

                # energy e[p, c] = sum_d key[p, c, d] * q[d]
                # DVE: broadcast multiply per half; reduction split between
                # DVE (one 3D reduce for half 0) and ACT (copy+accum, half 1)
                e_sb = smp.tile([P, CHUNKS], f32, tag="e_sb")
                q_bcast = q_rep[:][:, None, :].to_broadcast((P, HALF, D))
                prods = []
                for h in range(2):
                    prod = scp.tile([P, HALF, D], f32, tag=f"scr{h}")
                    nc.vector.tensor_tensor(prod[:], kts[h][:], q_bcast, Alu.mult)
                    prods.append(prod)
                nc.vector.tensor_reduce(
                    e_sb[:, 0:HALF], prods[0][:], mybir.AxisListType.X, Alu.add
                )
                for c in range(HALF):
                    trash = scp.tile([P, D], f32, tag="trash", bufs=1)
                    nc.scalar.activation(
                        trash[:],
                        prods[1][:, c, :],
                        mybir.ActivationFunctionType.Copy,
                        accum_out=e_sb[:, HALF + c : HALF + c + 1],
                    )

                # additive mask (-1e9 at t >= lens), then exp with fused row-sum
                em_sb = smp.tile([P, CHUNKS], f32, tag="em_sb")
                nc.vector.tensor_tensor(em_sb[:], e_sb[:], mask_sb[:], Alu.add)
                p_sb = smp.tile([P, CHUNKS], f32, tag="p_sb")
                part_sb = smp.tile([P, 1], f32, tag="part")
                nc.scalar.activation(
                    p_sb[:],
                    em_sb[:],
                    mybir.ActivationFunctionType.Exp,
                    bias=bias_sb[:],
                    scale=1.0,
                    accum_out=part_sb[:],
                )

                # total s = sum over partitions of part, r = 1/s
                s_ps = psp.tile([1, 1], f32, tag="ps")
                nc.tensor.matmul(s_ps[:], ones_col[:], part_sb[:])
                r_sb = smp.tile([1, 1], f32, tag="r_sb")
                nc.vector.reciprocal(r_sb[:], s_ps[:])

                # context[d] = r * sum_t p[t] * value[t, d] (accumulate over chunks)
                ctx_ps = psp.tile([1, D], f32, tag="ps")
                for c in range(CHUNKS):
                    nc.tensor.matmul(
                        ctx_ps[:],
                        p_sb[:, c : c + 1],
                        vts[c // HALF][:, c % HALF, :],
                        start=(c == 0),
                        stop=(c == CHUNKS - 1),
                    )
                ctx_sb = smp.tile([1, D], f32, tag="ctx_sb")
                nc.scalar.mul(ctx_sb[:], ctx_ps[:], mul=r_sb[:])
                nc.sync.dma_start(ctx_d[n : n + 1, :], ctx_sb[:])

                # attention out: transpose p to [c, p] layout, scale by r, store
                pT_ps = psp.tile([CHUNKS, P], f32, tag="ps")
                nc.tensor.transpose(pT_ps[:], p_sb[:], ident[:])
                rb_ps = psp.tile([CHUNKS, 1], f32, tag="ps")
                nc.tensor.matmul(rb_ps[:], ones_row[0:1, 0:CHUNKS], r_sb[:])
                rb_sb = smp.tile([CHUNKS, 1], f32, tag="rb_sb")
                nc.vector.tensor_copy(rb_sb[:], rb_ps[:])
                att_sb = smp.tile([CHUNKS, P], f32, tag="att_sb")
                nc.scalar.mul(att_sb[:], pT_ps[:], mul=rb_sb[:])
                nc.sync.dma_start(att3[n], att_sb[:])

    return nc


def _split_multiwaits(bir):
    """The walrus build in this container allows only ONE sync wait per
    instruction (setupSyncWait: 'Too many sync wait commands'). Tile attaches
    multiple waits to single instructions. Split the extras into standalone
    EventSemaphore wait instructions (same engine, placed immediately before)
    — semantically identical, just sequential waits."""
    ctr = 0
    for fn in bir["functions"]:
        for blk in fn["blocks"]:
            out = []
            for inst in blk["instructions"]:
                si = inst.get("sync_info")
                waits = (si or {}).get("on_wait") or []
                # raw-bytes ISA instructions encode waits in their payload;
                # rewriting sync_info desyncs the encoded length
                if len(waits) > 1 and not inst.get("instr"):
                    for w in waits[:-1]:
                        ctr += 1
                        pre = {
                            "name": f"I-mw{ctr}",
                            "opcode": "EventSemaphore",
                            "engine": inst["engine"],
                            "ins": [],
                            "outs": [],
                            "sync_info": {"on_update": [], "on_wait": [w]},
                        }
                        if "debug" in inst:
                            pre["debug"] = inst["debug"]
                        out.append(pre)
                    si["on_wait"] = [waits[-1]]
                out.append(inst)
            blk["instructions"] = out
    return bir


def _patch_json(nc):
    import json as _json

    orig = nc.to_json_bytes

    def patched():
        bir = _json.loads(orig())
        _split_multiwaits(bir)
        return _json.dumps(bir).encode()

    nc.to_json_bytes = patched
    return nc


def _get_nc():
    if "nc" not in _CACHE:
        _CACHE["nc"] = _patch_json(_build())
    return _CACHE["nc"]


def _make_in_maps(query, key, value, lens):
    query = np.ascontiguousarray(np.asarray(query, dtype=np.float32))
    key = np.ascontiguousarray(np.asarray(key, dtype=np.float32))
    value = np.ascontiguousarray(np.asarray(value, dtype=np.float32))
    lens = np.asarray(lens).astype(np.int64)

    # addmask[n, p, c] = 0 if c*128+p < lens[n] else -1e9
    t_idx = (np.arange(CHUNKS)[None, :] * P + np.arange(P)[:, None]).astype(np.int64)
    addmask = np.where(t_idx[None, :, :] < lens[:, None, None], 0.0, -1e9).astype(
        np.float32
    )
    ident = np.eye(P, dtype=np.float32)

    in_maps = []
    for i in range(NCORES):
        sl = slice(i * NB, (i + 1) * NB)
        in_maps.append(
            {
                "query": query[sl],
                "key": key[sl],
                "value": value[sl],
                "addmask": np.ascontiguousarray(addmask[sl]),
                "ident": ident,
            }
        )
    return in_maps


def _run(in_maps, trace=False, **kwargs):
    from concourse.bass_utils import run_bass_kernel_spmd

    nc = _get_nc()
    return run_bass_kernel_spmd(
        nc, in_maps, core_ids=list(range(NCORES)), trace=trace, **kwargs
    )


def kernel(query, key, value, lens):
    in_maps = _make_in_maps(query, key, value, lens)
    res = _run(in_maps, trace=False)
    ctx = np.concatenate([r["ctx_out"] for r in res.results], axis=0)
    att = np.concatenate([r["att_out"] for r in res.results], axis=0)
    return ctx, att


# revision 17
# speedup vs baseline: 1.3248x; 1.1374x over previous
"""Ragged-sequence attention kernel for 8 Trainium2 NeuronCores.

Problem: N=64 batches, T=2048, D=256.
  energy[n,t] = <key[n,t,:], query[n,:]>
  att = softmax(energy) masked to t < lens[n], renormalized
  context[n,:] = sum_t att[n,t] * value[n,t,:]

Math identity used: global softmax + mask + renormalize == masked softmax
(the global denominator cancels), so we compute exp(e - 20) with an additive
-1e9 mask folded into the energy, then normalize by the masked sum.

Sharding: pure data parallel, batch dim split 8 ways (8 batches/core).
"""

import numpy as np

N, T, D = 64, 2048, 256
NCORES = 8
NB = N // NCORES          # 8 batches per core
P = 128                   # SBUF partitions
CHUNKS = T // P           # 16 chunks of 128 timesteps
HALF = CHUNKS // 2        # key/value loaded in two half-DMAs

_CACHE = {}


def _build():
    import concourse.bass as bass
    import concourse.tile as tile
    from concourse import mybir

    f32 = mybir.dt.float32
    bf16 = mybir.dt.bfloat16
    Alu = mybir.AluOpType

    nc = bass.Bass()

    q_d = nc.declare_dram_parameter("query", [NB, D], f32, isOutput=False)
    k_d = nc.declare_dram_parameter("key", [NB, T, D], f32, isOutput=False)
    v_d = nc.declare_dram_parameter("value", [NB, T, D], f32, isOutput=False)
    m_d = nc.declare_dram_parameter("addmask", [NB, P, CHUNKS], f32, isOutput=False)
    id_d = nc.declare_dram_parameter("ident", [P, P], f32, isOutput=False)
    ctx_d = nc.declare_dram_parameter("ctx_out", [NB, D], f32, isOutput=True)
    att_d = nc.declare_dram_parameter("att_out", [NB, T], f32, isOutput=True)

    # t = c*128 + p  ->  partition p, chunk c
    key4 = k_d[:].rearrange("n (c p) d -> n p c d", p=P)
    val4 = v_d[:].rearrange("n (c p) d -> n p c d", p=P)
    att3 = att_d[:].rearrange("n (c p) -> n c p", p=P)

    with tile.TileContext(nc) as tc:
        with (
            tc.tile_pool(name="const", bufs=1) as constp,
            tc.tile_pool(name="kv", bufs=4) as kvp,
            tc.tile_pool(name="small", bufs=3) as smp,
            tc.tile_pool(name="scratch", bufs=2) as scp,
            tc.tile_pool(name="ps", bufs=8, space="PSUM") as psp,
        ):
            ones_col = constp.tile([P, 1], f32)
            nc.vector.memset(ones_col[:], 1.0)
            ones_row = constp.tile([1, P], f32)
            nc.vector.memset(ones_row[:], 1.0)
            ident = constp.tile([P, P], f32)
            nc.sync.dma_start(ident[:], id_d[:])
            bias_sb = constp.tile([P, 1], f32)
            nc.vector.memset(bias_sb[:], -20.0)

            for n in range(NB):
                # q broadcast across partitions via stride-0 DMA
                q_rep = smp.tile([P, D], f32, tag="q_rep")
                nc.sync.dma_start(q_rep[:], q_d[n : n + 1, :].to_broadcast((P, D)))
                mask_sb = smp.tile([P, CHUNKS], f32, tag="mask")
                nc.sync.dma_start(mask_sb[:], m_d[n])

                kts, vts = [], []
                for h in range(2):
                    kt = kvp.tile([P, HALF, D], f32, tag=f"kt{h}")
                    nc.sync.dma_start(kt[:], key4[n, :, h * HALF : (h + 1) * HALF, :])
                    kts.append(kt)
                for h in range(2):
                    # SWDGE casts f32->bf16 in flight; halves SBUF + PE passes
                    vt = kvp.tile([P, HALF, D], bf16, tag=f"vt{h}")
                    nc.gpsimd.dma_start(vt[:], val4[n, :, h * HALF : (h + 1) * HALF, :])
                    vts.append(vt)

                # energy e[p, c] = sum_d key[p, c, d] * q[d]
                # DVE: broadcast multiply per half; reduction split between
                # DVE (one 3D reduce for half 0) and ACT (copy+accum, half 1)
                e_sb = smp.tile([P, CHUNKS], f32, tag="e_sb")
                q_bcast = q_rep[:][:, None, :].to_broadcast((P, HALF, D))
                prods = []
                for h in range(2):
                    prod = scp.tile([P, HALF, D], f32, tag=f"scr{h}")
                    nc.vector.tensor_tensor(prod[:], kts[h][:], q_bcast, Alu.mult)
                    prods.append(prod)
                nc.vector.tensor_reduce(
                    e_sb[:, 0:HALF], prods[0][:], mybir.AxisListType.X, Alu.add
                )
                for c in range(HALF):
                    trash = scp.tile([P, D], f32, tag="trash", bufs=1)
                    nc.scalar.activation(
                        trash[:],
                        prods[1][:, c, :],
                        mybir.ActivationFunctionType.Copy,
                        accum_out=e_sb[:, HALF + c : HALF + c + 1],
                    )

                # additive mask (-1e9 at t >= lens), then exp with fused row-sum
                em_sb = smp.tile([P, CHUNKS], f32, tag="em_sb")
                nc.vector.tensor_tensor(em_sb[:], e_sb[:], mask_sb[:], Alu.add)
                p_sb = smp.tile([P, CHUNKS], f32, tag="p_sb")
                part_sb = smp.tile([P, 1], f32, tag="part")
                nc.scalar.activation(
                    p_sb[:],
                    em_sb[:],
                    mybir.ActivationFunctionType.Exp,
                    bias=bias_sb[:],
                    scale=1.0,
                    accum_out=part_sb[:],
                )

                # total s = sum over partitions of part, r = 1/s
                s_ps = psp.tile([1, 1], f32, tag="ps")
                nc.tensor.matmul(s_ps[:], ones_col[:], part_sb[:])
                r_sb = smp.tile([1, 1], f32, tag="r_sb")
                nc.vector.reciprocal(r_sb[:], s_ps[:])

                # context[d] = r * sum_t p[t] * value[t, d] (accumulate over chunks)
                # bf16 weights/values -> single-pass matmuls on PE
                p_bf = smp.tile([P, CHUNKS], bf16, tag="p_bf")
                nc.vector.tensor_copy(p_bf[:], p_sb[:])
                ctx_ps = psp.tile([1, D], f32, tag="ps")
                for c in range(CHUNKS):
                    nc.tensor.matmul(
                        ctx_ps[:],
                        p_bf[:, c : c + 1],
                        vts[c // HALF][:, c % HALF, :],
                        start=(c == 0),
                        stop=(c == CHUNKS - 1),
                    )
                ctx_sb = smp.tile([1, D], f32, tag="ctx_sb")
                nc.scalar.mul(ctx_sb[:], ctx_ps[:], mul=r_sb[:])
                nc.sync.dma_start(ctx_d[n : n + 1, :], ctx_sb[:])

                # attention out: transpose p to [c, p] layout, scale by r, store
                pT_ps = psp.tile([CHUNKS, P], f32, tag="ps")
                nc.tensor.transpose(pT_ps[:], p_sb[:], ident[:])
                rb_ps = psp.tile([CHUNKS, 1], f32, tag="ps")
                nc.tensor.matmul(rb_ps[:], ones_row[0:1, 0:CHUNKS], r_sb[:])
                rb_sb = smp.tile([CHUNKS, 1], f32, tag="rb_sb")
                nc.vector.tensor_copy(rb_sb[:], rb_ps[:])
                att_sb = smp.tile([CHUNKS, P], f32, tag="att_sb")
                nc.scalar.mul(att_sb[:], pT_ps[:], mul=rb_sb[:])
                nc.sync.dma_start(att3[n], att_sb[:])

    return nc


def _split_multiwaits(bir):
    """The walrus build in this container allows only ONE sync wait per
    instruction (setupSyncWait: 'Too many sync wait commands'). Tile attaches
    multiple waits to single instructions. Split the extras into standalone
    EventSemaphore wait instructions (same engine, placed immediately before)
    — semantically identical, just sequential waits."""
    ctr = 0
    for fn in bir["functions"]:
        for blk in fn["blocks"]:
            out = []
            for inst in blk["instructions"]:
                si = inst.get("sync_info")
                waits = (si or {}).get("on_wait") or []
                # raw-bytes ISA instructions encode waits in their payload;
                # rewriting sync_info desyncs the encoded length
                if len(waits) > 1 and not inst.get("instr"):
                    for w in waits[:-1]:
                        ctr += 1
                        pre = {
                            "name": f"I-mw{ctr}",
                            "opcode": "EventSemaphore",
                            "engine": inst["engine"],
                            "ins": [],
                            "outs": [],
                            "sync_info": {"on_update": [], "on_wait": [w]},
                        }
                        if "debug" in inst:
                            pre["debug"] = inst["debug"]
                        out.append(pre)
                    si["on_wait"] = [waits[-1]]
                out.append(inst)
            blk["instructions"] = out
    return bir


def _patch_json(nc):
    import json as _json

    orig = nc.to_json_bytes

    def patched():
        bir = _json.loads(orig())
        _split_multiwaits(bir)
        return _json.dumps(bir).encode()

    nc.to_json_bytes = patched
    return nc


def _get_nc():
    if "nc" not in _CACHE:
        _CACHE["nc"] = _patch_json(_build())
    return _CACHE["nc"]


def _make_in_maps(query, key, value, lens):
    query = np.ascontiguousarray(np.asarray(query, dtype=np.float32))
    key = np.ascontiguousarray(np.asarray(key, dtype=np.float32))
    value = np.ascontiguousarray(np.asarray(value, dtype=np.float32))
    lens = np.asarray(lens).astype(np.int64)

    # addmask[n, p, c] = 0 if c*128+p < lens[n] else -1e9
    t_idx = (np.arange(CHUNKS)[None, :] * P + np.arange(P)[:, None]).astype(np.int64)
    addmask = np.where(t_idx[None, :, :] < lens[:, None, None], 0.0, -1e9).astype(
        np.float32
    )
    ident = np.eye(P, dtype=np.float32)

    in_maps = []
    for i in range(NCORES):
        sl = slice(i * NB, (i + 1) * NB)
        in_maps.append(
            {
                "query": query[sl],
                "key": key[sl],
                "value": value[sl],
                "addmask": np.ascontiguousarray(addmask[sl]),
                "ident": ident,
            }
        )
    return in_maps


def _run(in_maps, trace=False, **kwargs):
    from concourse.bass_utils import run_bass_kernel_spmd

    nc = _get_nc()
    return run_bass_kernel_spmd(
        nc, in_maps, core_ids=list(range(NCORES)), trace=trace, **kwargs
    )


def kernel(query, key, value, lens):
    in_maps = _make_in_maps(query, key, value, lens)
    res = _run(in_maps, trace=False)
    ctx = np.concatenate([r["ctx_out"] for r in res.results], axis=0)
    att = np.concatenate([r["att_out"] for r in res.results], axis=0)
    return ctx, att


# revision 21
# speedup vs baseline: 1.4287x; 1.0785x over previous
"""Ragged-sequence attention kernel for 8 Trainium2 NeuronCores.

Problem: N=64 batches, T=2048, D=256.
  energy[n,t] = <key[n,t,:], query[n,:]>
  att = softmax(energy) masked to t < lens[n], renormalized
  context[n,:] = sum_t att[n,t] * value[n,t,:]

Math identity used: global softmax + mask + renormalize == masked softmax
(the global denominator cancels), so we compute exp(e - 20) with an additive
-1e9 mask folded into the energy, then normalize by the masked sum.

Sharding: pure data parallel, batch dim split 8 ways (8 batches/core).
"""

import numpy as np

N, T, D = 64, 2048, 256
NCORES = 8
NB = N // NCORES          # 8 batches per core
P = 128                   # SBUF partitions
CHUNKS = T // P           # 16 chunks of 128 timesteps
HALF = CHUNKS // 2        # key/value loaded in two half-DMAs

_CACHE = {}


def _build():
    import concourse.bass as bass
    import concourse.tile as tile
    from concourse import mybir

    f32 = mybir.dt.float32
    bf16 = mybir.dt.bfloat16
    Alu = mybir.AluOpType

    nc = bass.Bass()

    q_d = nc.declare_dram_parameter("query", [NB, D], f32, isOutput=False)
    k_d = nc.declare_dram_parameter("key", [NB, T, D], f32, isOutput=False)
    # value is uploaded pre-cast to bf16 (host side): halves its HBM read
    v_d = nc.declare_dram_parameter("value", [NB, T, D], bf16, isOutput=False)
    m_d = nc.declare_dram_parameter("addmask", [NB, P, CHUNKS], f32, isOutput=False)
    id_d = nc.declare_dram_parameter("ident", [P, P], f32, isOutput=False)
    ctx_d = nc.declare_dram_parameter("ctx_out", [NB, D], f32, isOutput=True)
    att_d = nc.declare_dram_parameter("att_out", [NB, T], f32, isOutput=True)

    # t = c*128 + p  ->  partition p, chunk c
    key4 = k_d[:].rearrange("n (c p) d -> n p c d", p=P)
    val4 = v_d[:].rearrange("n (c p) d -> n p c d", p=P)
    att3 = att_d[:].rearrange("n (c p) -> n c p", p=P)

    with tile.TileContext(nc) as tc:
        with (
            tc.tile_pool(name="const", bufs=1) as constp,
            tc.tile_pool(name="kv", bufs=5) as kvp,
            tc.tile_pool(name="small", bufs=3) as smp,
            tc.tile_pool(name="scratch", bufs=2) as scp,
            tc.tile_pool(name="ps", bufs=8, space="PSUM") as psp,
        ):
            ones_col = constp.tile([P, 1], f32)
            nc.vector.memset(ones_col[:], 1.0)
            ones_row = constp.tile([1, P], f32)
            nc.vector.memset(ones_row[:], 1.0)
            ident = constp.tile([P, P], f32)
            nc.sync.dma_start(ident[:], id_d[:])
            bias_sb = constp.tile([P, 1], f32)
            nc.vector.memset(bias_sb[:], -20.0)

            for n in range(NB):
                # q broadcast across partitions via stride-0 DMA
                q_rep = smp.tile([P, D], f32, tag="q_rep")
                nc.sync.dma_start(q_rep[:], q_d[n : n + 1, :].to_broadcast((P, D)))
                mask_sb = smp.tile([P, CHUNKS], f32, tag="mask")
                nc.sync.dma_start(mask_sb[:], m_d[n])

                kts, vts = [], []
                for h in range(2):
                    kt = kvp.tile([P, HALF, D], f32, tag=f"kt{h}")
                    nc.sync.dma_start(kt[:], key4[n, :, h * HALF : (h + 1) * HALF, :])
                    kts.append(kt)
                for h in range(2):
                    vt = kvp.tile([P, HALF, D], bf16, tag=f"vt{h}")
                    nc.sync.dma_start(vt[:], val4[n, :, h * HALF : (h + 1) * HALF, :])
                    vts.append(vt)

                # energy e[p, c] = sum_d key[p, c, d] * q[d]
                # DVE: broadcast multiply per half; reduction split between
                # DVE (one 3D reduce for half 0) and ACT (copy+accum, half 1)
                e_sb = smp.tile([P, CHUNKS], f32, tag="e_sb")
                q_bcast = q_rep[:][:, None, :].to_broadcast((P, HALF, D))
                prods = []
                for h in range(2):
                    prod = scp.tile([P, HALF, D], f32, tag=f"scr{h}")
                    nc.vector.tensor_tensor(prod[:], kts[h][:], q_bcast, Alu.mult)
                    prods.append(prod)
                nc.vector.tensor_reduce(
                    e_sb[:, 0:HALF], prods[0][:], mybir.AxisListType.X, Alu.add
                )
                for c in range(HALF):
                    trash = scp.tile([P, D], f32, tag="trash", bufs=1)
                    nc.scalar.activation(
                        trash[:],
                        prods[1][:, c, :],
                        mybir.ActivationFunctionType.Copy,
                        accum_out=e_sb[:, HALF + c : HALF + c + 1],
                    )

                # additive mask (-1e9 at t >= lens), then exp with fused row-sum
                em_sb = smp.tile([P, CHUNKS], f32, tag="em_sb")
                nc.vector.tensor_tensor(em_sb[:], e_sb[:], mask_sb[:], Alu.add)
                p_sb = smp.tile([P, CHUNKS], f32, tag="p_sb")
                part_sb = smp.tile([P, 1], f32, tag="part")
                nc.scalar.activation(
                    p_sb[:],
                    em_sb[:],
                    mybir.ActivationFunctionType.Exp,
                    bias=bias_sb[:],
                    scale=1.0,
                    accum_out=part_sb[:],
                )

                # total s = sum over partitions of part, r = 1/s
                s_ps = psp.tile([1, 1], f32, tag="ps")
                nc.tensor.matmul(s_ps[:], ones_col[:], part_sb[:])
                r_sb = smp.tile([1, 1], f32, tag="r_sb")
                nc.vector.reciprocal(r_sb[:], s_ps[:])

                # context[d] = r * sum_t p[t] * value[t, d] (accumulate over chunks)
                # bf16 weights/values -> single-pass matmuls on PE
                p_bf = smp.tile([P, CHUNKS], bf16, tag="p_bf")
                nc.vector.tensor_copy(p_bf[:], p_sb[:])
                ctx_ps = psp.tile([1, D], f32, tag="ps")
                for c in range(CHUNKS):
                    nc.tensor.matmul(
                        ctx_ps[:],
                        p_bf[:, c : c + 1],
                        vts[c // HALF][:, c % HALF, :],
                        start=(c == 0),
                        stop=(c == CHUNKS - 1),
                    )
                ctx_sb = smp.tile([1, D], f32, tag="ctx_sb")
                nc.scalar.mul(ctx_sb[:], ctx_ps[:], mul=r_sb[:])
                nc.sync.dma_start(ctx_d[n : n + 1, :], ctx_sb[:])

                # attention out: transpose p to [c, p] layout, scale by r, store
                pT_ps = psp.tile([CHUNKS, P], f32, tag="ps")
                nc.tensor.transpose(pT_ps[:], p_sb[:], ident[:])
                rb_ps = psp.tile([CHUNKS, 1], f32, tag="ps")
                nc.tensor.matmul(rb_ps[:], ones_row[0:1, 0:CHUNKS], r_sb[:])
                rb_sb = smp.tile([CHUNKS, 1], f32, tag="rb_sb")
                nc.vector.tensor_copy(rb_sb[:], rb_ps[:])
                att_sb = smp.tile([CHUNKS, P], f32, tag="att_sb")
                nc.scalar.mul(att_sb[:], pT_ps[:], mul=rb_sb[:])
                nc.sync.dma_start(att3[n], att_sb[:])

    return nc


def _split_multiwaits(bir):
    """The walrus build in this container allows only ONE sync wait per
    instruction (setupSyncWait: 'Too many sync wait commands'). Tile attaches
    multiple waits to single instructions. Split the extras into standalone
    EventSemaphore wait instructions (same engine, placed immediately before)
    — semantically identical, just sequential waits."""
    ctr = 0
    for fn in bir["functions"]:
        for blk in fn["blocks"]:
            out = []
            for inst in blk["instructions"]:
                si = inst.get("sync_info")
                waits = (si or {}).get("on_wait") or []
                # raw-bytes ISA instructions encode waits in their payload;
                # rewriting sync_info desyncs the encoded length
                if len(waits) > 1 and not inst.get("instr"):
                    for w in waits[:-1]:
                        ctr += 1
                        pre = {
                            "name": f"I-mw{ctr}",
                            "opcode": "EventSemaphore",
                            "engine": inst["engine"],
                            "ins": [],
                            "outs": [],
                            "sync_info": {"on_update": [], "on_wait": [w]},
                        }
                        if "debug" in inst:
                            pre["debug"] = inst["debug"]
                        out.append(pre)
                    si["on_wait"] = [waits[-1]]
                out.append(inst)
            blk["instructions"] = out
    return bir


def _patch_json(nc):
    import json as _json

    orig = nc.to_json_bytes

    def patched():
        bir = _json.loads(orig())
        _split_multiwaits(bir)
        return _json.dumps(bir).encode()

    nc.to_json_bytes = patched
    return nc


def _get_nc():
    if "nc" not in _CACHE:
        _CACHE["nc"] = _patch_json(_build())
    return _CACHE["nc"]


def _make_in_maps(query, key, value, lens):
    import ml_dtypes

    query = np.ascontiguousarray(np.asarray(query, dtype=np.float32))
    key = np.ascontiguousarray(np.asarray(key, dtype=np.float32))
    value = np.ascontiguousarray(
        np.asarray(value, dtype=np.float32).astype(ml_dtypes.bfloat16)
    )
    lens = np.asarray(lens).astype(np.int64)

    # addmask[n, p, c] = 0 if c*128+p < lens[n] else -1e9
    t_idx = (np.arange(CHUNKS)[None, :] * P + np.arange(P)[:, None]).astype(np.int64)
    addmask = np.where(t_idx[None, :, :] < lens[:, None, None], 0.0, -1e9).astype(
        np.float32
    )
    ident = np.eye(P, dtype=np.float32)

    in_maps = []
    for i in range(NCORES):
        sl = slice(i * NB, (i + 1) * NB)
        in_maps.append(
            {
                "query": query[sl],
                "key": key[sl],
                "value": value[sl],
                "addmask": np.ascontiguousarray(addmask[sl]),
                "ident": ident,
            }
        )
    return in_maps


def _run(in_maps, trace=False, **kwargs):
    from concourse.bass_utils import run_bass_kernel_spmd

    nc = _get_nc()
    return run_bass_kernel_spmd(
        nc, in_maps, core_ids=list(range(NCORES)), trace=trace, **kwargs
    )


def kernel(query, key, value, lens):
    in_maps = _make_in_maps(query, key, value, lens)
    res = _run(in_maps, trace=False)
    ctx = np.concatenate([r["ctx_out"] for r in res.results], axis=0)
    att = np.concatenate([r["att_out"] for r in res.results], axis=0)
    return ctx, att


# revision 23
# speedup vs baseline: 1.4756x; 1.0328x over previous
"""Ragged-sequence attention kernel for 8 Trainium2 NeuronCores.

Problem: N=64 batches, T=2048, D=256.
  energy[n,t] = <key[n,t,:], query[n,:]>
  att = softmax(energy) masked to t < lens[n], renormalized
  context[n,:] = sum_t att[n,t] * value[n,t,:]

Math identity used: global softmax + mask + renormalize == masked softmax
(the global denominator cancels), so we compute exp(e - 20) with an additive
-1e9 mask folded into the energy, then normalize by the masked sum.

Sharding: pure data parallel, batch dim split 8 ways (8 batches/core).
"""

import numpy as np

N, T, D = 64, 2048, 256
NCORES = 8
NB = N // NCORES          # 8 batches per core
P = 128                   # SBUF partitions
CHUNKS = T // P           # 16 chunks of 128 timesteps
HALF = CHUNKS // 2        # key/value loaded in two half-DMAs

_CACHE = {}


def _build():
    import concourse.bass as bass
    import concourse.tile as tile
    from concourse import mybir

    f32 = mybir.dt.float32
    bf16 = mybir.dt.bfloat16
    Alu = mybir.AluOpType

    nc = bass.Bass()

    q_d = nc.declare_dram_parameter("query", [NB, D], f32, isOutput=False)
    k_d = nc.declare_dram_parameter("key", [NB, T, D], f32, isOutput=False)
    # value is uploaded pre-cast to bf16 (host side): halves its HBM read
    v_d = nc.declare_dram_parameter("value", [NB, T, D], bf16, isOutput=False)
    m_d = nc.declare_dram_parameter("addmask", [NB, P, CHUNKS], f32, isOutput=False)
    id_d = nc.declare_dram_parameter("ident", [P, P], f32, isOutput=False)
    ctx_d = nc.declare_dram_parameter("ctx_out", [NB, D], f32, isOutput=True)
    att_d = nc.declare_dram_parameter("att_out", [NB, T], f32, isOutput=True)

    # t = c*128 + p  ->  partition p, chunk c
    key4 = k_d[:].rearrange("n (c p) d -> n p c d", p=P)
    val4 = v_d[:].rearrange("n (c p) d -> n p c d", p=P)
    att3 = att_d[:].rearrange("n (c p) -> n c p", p=P)

    with tile.TileContext(nc) as tc:
        with (
            tc.tile_pool(name="const", bufs=1) as constp,
            tc.tile_pool(name="kv", bufs=5) as kvp,
            tc.tile_pool(name="small", bufs=3) as smp,
            tc.tile_pool(name="scratch", bufs=2) as scp,
            tc.tile_pool(name="ps", bufs=8, space="PSUM") as psp,
        ):
            ones_col = constp.tile([P, 1], f32)
            nc.vector.memset(ones_col[:], 1.0)
            ones_row = constp.tile([1, P], f32)
            nc.vector.memset(ones_row[:], 1.0)
            ident = constp.tile([P, P], f32)
            nc.sync.dma_start(ident[:], id_d[:])
            bias_sb = constp.tile([P, 1], f32)
            nc.vector.memset(bias_sb[:], -20.0)

            for n in range(NB):
                # q broadcast across partitions via stride-0 DMA
                q_rep = smp.tile([P, D], f32, tag="q_rep")
                nc.sync.dma_start(q_rep[:], q_d[n : n + 1, :].to_broadcast((P, D)))
                mask_sb = smp.tile([P, CHUNKS], f32, tag="mask")
                nc.sync.dma_start(mask_sb[:], m_d[n])

                kts, vts = [], []
                for h in range(2):
                    kt = kvp.tile([P, HALF, D], f32, tag=f"kt{h}")
                    nc.sync.dma_start(kt[:], key4[n, :, h * HALF : (h + 1) * HALF, :])
                    kts.append(kt)
                for h in range(2):
                    vt = kvp.tile([P, HALF, D], bf16, tag=f"vt{h}")
                    nc.sync.dma_start(vt[:], val4[n, :, h * HALF : (h + 1) * HALF, :])
                    vts.append(vt)

                # energy e[p, c] = sum_d key[p, c, d] * q[d]
                # DVE: broadcast multiply per half; reduction split between
                # DVE (one 3D reduce for half 0) and ACT (copy+accum, half 1)
                e_sb = smp.tile([P, CHUNKS], f32, tag="e_sb")
                q_bcast = q_rep[:][:, None, :].to_broadcast((P, HALF, D))
                prods = []
                for h in range(2):
                    prod = scp.tile([P, HALF, D], f32, tag=f"scr{h}")
                    nc.vector.tensor_tensor(prod[:], kts[h][:], q_bcast, Alu.mult)
                    prods.append(prod)
                nc.vector.tensor_reduce(
                    e_sb[:, 0:HALF], prods[0][:], mybir.AxisListType.X, Alu.add
                )
                for c in range(HALF):
                    trash = scp.tile([P, D], f32, tag="trash", bufs=1)
                    nc.scalar.activation(
                        trash[:],
                        prods[1][:, c, :],
                        mybir.ActivationFunctionType.Copy,
                        accum_out=e_sb[:, HALF + c : HALF + c + 1],
                    )

                # additive mask (-1e9 at t >= lens), then exp with fused row-sum
                em_sb = smp.tile([P, CHUNKS], f32, tag="em_sb")
                nc.vector.tensor_tensor(em_sb[:], e_sb[:], mask_sb[:], Alu.add)
                p_sb = smp.tile([P, CHUNKS], f32, tag="p_sb")
                part_sb = smp.tile([P, 1], f32, tag="part")
                nc.scalar.activation(
                    p_sb[:],
                    em_sb[:],
                    mybir.ActivationFunctionType.Exp,
                    bias=bias_sb[:],
                    scale=1.0,
                    accum_out=part_sb[:],
                )

                # total s = sum over partitions of part, r = 1/s
                s_ps = psp.tile([1, 1], f32, tag="ps")
                nc.tensor.matmul(s_ps[:], ones_col[:], part_sb[:])
                r_sb = smp.tile([1, 1], f32, tag="r_sb")
                nc.vector.reciprocal(r_sb[:], s_ps[:])

                # context[d] = r * sum_t p[t] * value[t, d] (accumulate over chunks)
                # bf16 weights/values -> single-pass matmuls on PE
                p_bf = smp.tile([P, CHUNKS], bf16, tag="p_bf")
                nc.vector.tensor_copy(p_bf[:], p_sb[:])
                ctx_ps = psp.tile([1, D], f32, tag="ps")
                for c in range(CHUNKS):
                    nc.tensor.matmul(
                        ctx_ps[:],
                        p_bf[:, c : c + 1],
                        vts[c // HALF][:, c % HALF, :],
                        start=(c == 0),
                        stop=(c == CHUNKS - 1),
                    )
                ctx_sb = smp.tile([1, D], f32, tag="ctx_sb")
                nc.scalar.mul(ctx_sb[:], ctx_ps[:], mul=r_sb[:])
                nc.scalar.dma_start(ctx_d[n : n + 1, :], ctx_sb[:])

                # attention out: transpose p to [c, p] layout, scale by r, store
                pT_ps = psp.tile([CHUNKS, P], f32, tag="ps")
                nc.tensor.transpose(pT_ps[:], p_sb[:], ident[:])
                rb_ps = psp.tile([CHUNKS, 1], f32, tag="ps")
                nc.tensor.matmul(rb_ps[:], ones_row[0:1, 0:CHUNKS], r_sb[:])
                rb_sb = smp.tile([CHUNKS, 1], f32, tag="rb_sb")
                nc.vector.tensor_copy(rb_sb[:], rb_ps[:])
                att_sb = smp.tile([CHUNKS, P], f32, tag="att_sb")
                nc.scalar.mul(att_sb[:], pT_ps[:], mul=rb_sb[:])
                nc.scalar.dma_start(att3[n], att_sb[:])

    return nc


def _split_multiwaits(bir):
    """The walrus build in this container allows only ONE sync wait per
    instruction (setupSyncWait: 'Too many sync wait commands'). Tile attaches
    multiple waits to single instructions. Split the extras into standalone
    EventSemaphore wait instructions (same engine, placed immediately before)
    — semantically identical, just sequential waits."""
    ctr = 0
    for fn in bir["functions"]:
        for blk in fn["blocks"]:
            out = []
            for inst in blk["instructions"]:
                si = inst.get("sync_info")
                waits = (si or {}).get("on_wait") or []
                # raw-bytes ISA instructions encode waits in their payload;
                # rewriting sync_info desyncs the encoded length
                if len(waits) > 1 and not inst.get("instr"):
                    for w in waits[:-1]:
                        ctr += 1
                        pre = {
                            "name": f"I-mw{ctr}",
                            "opcode": "EventSemaphore",
                            "engine": inst["engine"],
                            "ins": [],
                            "outs": [],
                            "sync_info": {"on_update": [], "on_wait": [w]},
                        }
                        if "debug" in inst:
                            pre["debug"] = inst["debug"]
                        out.append(pre)
                    si["on_wait"] = [waits[-1]]
                out.append(inst)
            blk["instructions"] = out
    return bir


def _patch_json(nc):
    import json as _json

    orig = nc.to_json_bytes

    def patched():
        bir = _json.loads(orig())
        _split_multiwaits(bir)
        return _json.dumps(bir).encode()

    nc.to_json_bytes = patched
    return nc


def _get_nc():
    if "nc" not in _CACHE:
        _CACHE["nc"] = _patch_json(_build())
    return _CACHE["nc"]


def _make_in_maps(query, key, value, lens):
    import ml_dtypes

    query = np.ascontiguousarray(np.asarray(query, dtype=np.float32))
    key = np.ascontiguousarray(np.asarray(key, dtype=np.float32))
    value = np.ascontiguousarray(
        np.asarray(value, dtype=np.float32).astype(ml_dtypes.bfloat16)
    )
    lens = np.asarray(lens).astype(np.int64)

    # addmask[n, p, c] = 0 if c*128+p < lens[n] else -1e9
    t_idx = (np.arange(CHUNKS)[None, :] * P + np.arange(P)[:, None]).astype(np.int64)
    addmask = np.where(t_idx[None, :, :] < lens[:, None, None], 0.0, -1e9).astype(
        np.float32
    )
    ident = np.eye(P, dtype=np.float32)

    in_maps = []
    for i in range(NCORES):
        sl = slice(i * NB, (i + 1) * NB)
        in_maps.append(
            {
                "query": query[sl],
                "key": key[sl],
                "value": value[sl],
                "addmask": np.ascontiguousarray(addmask[sl]),
                "ident": ident,
            }
        )
    return in_maps


def _run(in_maps, trace=False, **kwargs):
    from concourse.bass_utils import run_bass_kernel_spmd

    nc = _get_nc()
    return run_bass_kernel_spmd(
        nc, in_maps, core_ids=list(range(NCORES)), trace=trace, **kwargs
    )


def kernel(query, key, value, lens):
    in_maps = _make_in_maps(query, key, value, lens)
    res = _run(in_maps, trace=False)
    ctx = np.concatenate([r["ctx_out"] for r in res.results], axis=0)
    att = np.concatenate([r["att_out"] for r in res.results], axis=0)
    return ctx, att


# revision 27
# speedup vs baseline: 1.5023x; 1.0181x over previous
"""Ragged-sequence attention kernel for 8 Trainium2 NeuronCores.

Problem: N=64 batches, T=2048, D=256.
  energy[n,t] = <key[n,t,:], query[n,:]>
  att = softmax(energy) masked to t < lens[n], renormalized
  context[n,:] = sum_t att[n,t] * value[n,t,:]

Math identity used: global softmax + mask + renormalize == masked softmax
(the global denominator cancels), so we compute exp(e - 20) with an additive
-1e9 mask folded into the energy, then normalize by the masked sum.

Sharding: pure data parallel, batch dim split 8 ways (8 batches/core).
"""

import numpy as np

N, T, D = 64, 2048, 256
NCORES = 8
NB = N // NCORES          # 8 batches per core
P = 128                   # SBUF partitions
CHUNKS = T // P           # 16 chunks of 128 timesteps
HALF = CHUNKS // 2        # key/value loaded in two half-DMAs

_CACHE = {}


def _build():
    import concourse.bass as bass
    import concourse.tile as tile
    from concourse import mybir

    f32 = mybir.dt.float32
    bf16 = mybir.dt.bfloat16
    Alu = mybir.AluOpType

    nc = bass.Bass()

    q_d = nc.declare_dram_parameter("query", [NB, D], f32, isOutput=False)
    k_d = nc.declare_dram_parameter("key", [NB, T, D], f32, isOutput=False)
    # value is uploaded pre-cast to bf16 (host side): halves its HBM read
    v_d = nc.declare_dram_parameter("value", [NB, T, D], bf16, isOutput=False)
    m_d = nc.declare_dram_parameter("addmask", [NB, P, CHUNKS], f32, isOutput=False)
    ctx_d = nc.declare_dram_parameter("ctx_out", [NB, D], f32, isOutput=True)
    att_d = nc.declare_dram_parameter("att_out", [NB, T], f32, isOutput=True)

    # t = p*16 + c -> partition p holds 16 consecutive timesteps; DRAM runs
    # are 8-16 KB contiguous per partition (fast DMA) and the attention
    # output stores directly from [p, c] layout (no transpose needed)
    key4 = k_d[:].rearrange("n (p c) d -> n p c d", c=CHUNKS)
    val4 = v_d[:].rearrange("n (p c) d -> n p c d", c=CHUNKS)
    att3 = att_d[:].rearrange("n (p c) -> n p c", c=CHUNKS)

    with tile.TileContext(nc) as tc:
        with (
            tc.tile_pool(name="const", bufs=1) as constp,
            tc.tile_pool(name="kv", bufs=5) as kvp,
            tc.tile_pool(name="small", bufs=3) as smp,
            tc.tile_pool(name="scratch", bufs=2) as scp,
            tc.tile_pool(name="ps", bufs=8, space="PSUM") as psp,
        ):
            ones_col = constp.tile([P, 1], f32)
            nc.vector.memset(ones_col[:], 1.0)
            ones_row = constp.tile([1, P], f32)
            nc.vector.memset(ones_row[:], 1.0)
            bias_sb = constp.tile([P, 1], f32)
            nc.vector.memset(bias_sb[:], -20.0)

            for n in range(NB):
                # q broadcast across partitions via stride-0 DMA
                q_rep = smp.tile([P, D], f32, tag="q_rep")
                nc.sync.dma_start(q_rep[:], q_d[n : n + 1, :].to_broadcast((P, D)))
                mask_sb = smp.tile([P, CHUNKS], f32, tag="mask")
                nc.sync.dma_start(mask_sb[:], m_d[n])

                kts, vts = [], []
                for h in range(2):
                    kt = kvp.tile([P, HALF, D], f32, tag=f"kt{h}")
                    nc.sync.dma_start(kt[:], key4[n, :, h * HALF : (h + 1) * HALF, :])
                    kts.append(kt)
                for h in range(2):
                    vt = kvp.tile([P, HALF, D], bf16, tag=f"vt{h}")
                    nc.sync.dma_start(vt[:], val4[n, :, h * HALF : (h + 1) * HALF, :])
                    vts.append(vt)

                # energy e[p, c] = sum_d key[p, c, d] * q[d]
                # DVE: broadcast multiply per half; reduction split between
                # DVE (one 3D reduce for half 0) and ACT (copy+accum, half 1)
                e_sb = smp.tile([P, CHUNKS], f32, tag="e_sb")
                q_bcast = q_rep[:][:, None, :].to_broadcast((P, HALF, D))
                prods = []
                for h in range(2):
                    prod = scp.tile([P, HALF, D], f32, tag=f"scr{h}")
                    nc.vector.tensor_tensor(prod[:], kts[h][:], q_bcast, Alu.mult)
                    prods.append(prod)
                nc.vector.tensor_reduce(
                    e_sb[:, 0:HALF], prods[0][:], mybir.AxisListType.X, Alu.add
                )
                for c in range(HALF):
                    trash = scp.tile([P, D], f32, tag="trash", bufs=1)
                    nc.scalar.activation(
                        trash[:],
                        prods[1][:, c, :],
                        mybir.ActivationFunctionType.Copy,
                        accum_out=e_sb[:, HALF + c : HALF + c + 1],
                    )

                # additive mask (-1e9 at t >= lens), then exp with fused row-sum
                em_sb = smp.tile([P, CHUNKS], f32, tag="em_sb")
                nc.vector.tensor_tensor(em_sb[:], e_sb[:], mask_sb[:], Alu.add)
                p_sb = smp.tile([P, CHUNKS], f32, tag="p_sb")
                part_sb = smp.tile([P, 1], f32, tag="part")
                nc.scalar.activation(
                    p_sb[:],
                    em_sb[:],
                    mybir.ActivationFunctionType.Exp,
                    bias=bias_sb[:],
                    scale=1.0,
                    accum_out=part_sb[:],
                )

                # total s = sum over partitions of part, r = 1/s
                s_ps = psp.tile([1, 1], f32, tag="ps")
                nc.tensor.matmul(s_ps[:], ones_col[:], part_sb[:])
                r_sb = smp.tile([1, 1], f32, tag="r_sb")
                nc.vector.reciprocal(r_sb[:], s_ps[:])

                # context[d] = r * sum_t p[t] * value[t, d] (accumulate over chunks)
                # bf16 weights/values -> single-pass matmuls on PE
                p_bf = smp.tile([P, CHUNKS], bf16, tag="p_bf")
                nc.vector.tensor_copy(p_bf[:], p_sb[:])
                ctx_ps = psp.tile([1, D], f32, tag="ps")
                for c in range(CHUNKS):
                    nc.tensor.matmul(
                        ctx_ps[:],
                        p_bf[:, c : c + 1],
                        vts[c // HALF][:, c % HALF, :],
                        start=(c == 0),
                        stop=(c == CHUNKS - 1),
                    )
                ctx_sb = smp.tile([1, D], f32, tag="ctx_sb")
                nc.scalar.mul(ctx_sb[:], ctx_ps[:], mul=r_sb[:])
                nc.scalar.dma_start(ctx_d[n : n + 1, :], ctx_sb[:])

                # attention out: att[p, c] = p[p, c] * r, stored directly
                rb_ps = psp.tile([P, 1], f32, tag="ps")
                nc.tensor.matmul(rb_ps[:], ones_row[:], r_sb[:])
                rb_sb = smp.tile([P, 1], f32, tag="rb_sb")
                nc.vector.tensor_copy(rb_sb[:], rb_ps[:])
                att_sb = smp.tile([P, CHUNKS], f32, tag="att_sb")
                nc.scalar.mul(att_sb[:], p_sb[:], mul=rb_sb[:])
                nc.scalar.dma_start(att3[n], att_sb[:])

    return nc


def _split_multiwaits(bir):
    """The walrus build in this container allows only ONE sync wait per
    instruction (setupSyncWait: 'Too many sync wait commands'). Tile attaches
    multiple waits to single instructions. Split the extras into standalone
    EventSemaphore wait instructions (same engine, placed immediately before)
    — semantically identical, just sequential waits."""
    ctr = 0
    for fn in bir["functions"]:
        for blk in fn["blocks"]:
            out = []
            for inst in blk["instructions"]:
                si = inst.get("sync_info")
                waits = (si or {}).get("on_wait") or []
                # raw-bytes ISA instructions encode waits in their payload;
                # rewriting sync_info desyncs the encoded length
                if len(waits) > 1 and not inst.get("instr"):
                    for w in waits[:-1]:
                        ctr += 1
                        pre = {
                            "name": f"I-mw{ctr}",
                            "opcode": "EventSemaphore",
                            "engine": inst["engine"],
                            "ins": [],
                            "outs": [],
                            "sync_info": {"on_update": [], "on_wait": [w]},
                        }
                        if "debug" in inst:
                            pre["debug"] = inst["debug"]
                        out.append(pre)
                    si["on_wait"] = [waits[-1]]
                out.append(inst)
            blk["instructions"] = out
    return bir


def _patch_json(nc):
    import json as _json

    orig = nc.to_json_bytes

    def patched():
        bir = _json.loads(orig())
        _split_multiwaits(bir)
        return _json.dumps(bir).encode()

    nc.to_json_bytes = patched
    return nc


def _get_nc():
    if "nc" not in _CACHE:
        _CACHE["nc"] = _patch_json(_build())
    return _CACHE["nc"]


def _make_in_maps(query, key, value, lens):
    import ml_dtypes

    query = np.ascontiguousarray(np.asarray(query, dtype=np.float32))
    key = np.ascontiguousarray(np.asarray(key, dtype=np.float32))
    value = np.ascontiguousarray(
        np.asarray(value, dtype=np.float32).astype(ml_dtypes.bfloat16)
    )
    lens = np.asarray(lens).astype(np.int64)

    # addmask[n, p, c] = 0 if p*16+c < lens[n] else -1e9
    t_idx = np.arange(T).reshape(P, CHUNKS)
    addmask = np.where(t_idx[None, :, :] < lens[:, None, None], 0.0, -1e9).astype(
        np.float32
    )

    in_maps = []
    for i in range(NCORES):
        sl = slice(i * NB, (i + 1) * NB)
        in_maps.append(
            {
                "query": query[sl],
                "key": key[sl],
                "value": value[sl],
                "addmask": np.ascontiguousarray(addmask[sl]),
            }
        )
    return in_maps


def _run(in_maps, trace=False, **kwargs):
    from concourse.bass_utils import run_bass_kernel_spmd

    nc = _get_nc()
    return run_bass_kernel_spmd(
        nc, in_maps, core_ids=list(range(NCORES)), trace=trace, **kwargs
    )


def kernel(query, key, value, lens):
    in_maps = _make_in_maps(query, key, value, lens)
    res = _run(in_maps, trace=False)
    ctx = np.concatenate([r["ctx_out"] for r in res.results], axis=0)
    att = np.concatenate([r["att_out"] for r in res.results], axis=0)
    return ctx, att
